# revision 25
# baseline (speedup 1.0000x reference)
"""Trainium2 Bass kernel for nn_Attention3D (spatial-reduction attention).

Sharding: 8 cores = 4 batches x 2 head-groups (4 heads each).
Each core computes, for its (batch b, heads 4g..4g+3):
  qkv = x_b @ Wqkv_slice            (feature-major qkvT layout, fp32r matmuls)
  SR branch: depthwise 2x2/2 conv -> LayerNorm -> linear  => x2 [64, 1024]
  scores_k/v = softmax_n(x2 . k/v)  (exp on ACT; denominators via ones-column)
  k_c/v_c    = softmax-weighted pooling of k/v
  attn       = softmax_m(q . k_c * scale); out = attn . v_c
  partial    = out_heads @ Wproj_rows    (host sums the two head-group partials)

Host-side: slice inputs per core, run SPMD on 8 NeuronCores, sum partials + b_proj.
"""
import numpy as np
from contextlib import ExitStack

import concourse.bass as bass
import concourse.tile as tile
from concourse import bacc, mybir
from concourse import bass_utils
from concourse.masks import make_identity
from concourse.alu_op_type import AluOpType

# problem constants
B, N, C, H, HD, SR = 4, 4096, 512, 8, 64, 2
M = 1024              # pooled tokens: (64/2)^2
HL = 4                # heads per core
SCALE = HD ** -0.5
EPS = 1e-5

F32 = mybir.dt.float32
F32R = mybir.dt.float32r
AF = mybir.ActivationFunctionType
OP = AluOpType

_CACHE = {}


def _build_nc():
    nc = bacc.Bacc("TRN2", target_bir_lowering=False, debug=False, num_devices=8)

    # ---- DRAM I/O (per-core shard shapes) ----
    x_d = nc.dram_tensor("x", [N, C], F32R, kind="ExternalInput").ap()
    wqkv_d = nc.dram_tensor("wqkv", [C, 3 * HL * HD], F32R, kind="ExternalInput").ap()
    wconv_d = nc.dram_tensor("wconv", [C, 4], F32, kind="ExternalInput").ap()
    bconv_d = nc.dram_tensor("bconv", [C, 1], F32, kind="ExternalInput").ap()
    gam_d = nc.dram_tensor("gam", [C, 1], F32, kind="ExternalInput").ap()
    bet_d = nc.dram_tensor("bet", [C, 1], F32, kind="ExternalInput").ap()
    wlin_d = nc.dram_tensor("wlin", [C, 128], F32R, kind="ExternalInput").ap()   # duplicated cols
    blin_d = nc.dram_tensor("blin", [128, 1], F32, kind="ExternalInput").ap()    # duplicated rows
    wproj_d = nc.dram_tensor("wproj", [HL * HD, C], F32R, kind="ExternalInput").ap()
    out_d = nc.dram_tensor("out", [N, C], F32, kind="ExternalOutput").ap()
    # scratch for normalized per-head attention outputs, feature-major [d, h, n]
    oT_d = nc.dram_tensor("oT", [HD, HL, N], F32R, kind="Internal").ap()

    with tile.TileContext(nc) as tc, \
         nc.allow_low_precision("fp32r operands are intentional"), \
         ExitStack() as ctx:
        # ---------------- persistent pools ----------------
        singles = ctx.enter_context(tc.tile_pool(name="singles", bufs=1))
        main = ctx.enter_context(tc.tile_pool(name="main", bufs=1))

        ident_f = singles.tile([128, 128], F32)
        make_identity(nc, ident_f)
        ident = singles.tile([128, 128], F32R)
        nc.vector.tensor_copy(ident, ident_f)

        ones_col_f = singles.tile([128, 1], F32)
        nc.vector.memset(ones_col_f, 1.0)
        ones_col_r = singles.tile([128, 1], F32R)
        nc.vector.tensor_copy(ones_col_r, ones_col_f)
        ones_row_r = singles.tile([1, 128], F32R)
        nc.vector.tensor_copy(ones_row_r, ident_f[0:1, :])  # placeholder; fixed below
        ones_row_f = singles.tile([1, 128], F32)
        nc.vector.memset(ones_row_f, 1.0)
        nc.vector.tensor_copy(ones_row_r, ones_row_f)
        ones32_f = singles.tile([128, 32], F32)
        nc.vector.memset(ones32_f, 1.0)
        ones64_f = singles.tile([128, 64], F32)
        nc.vector.memset(ones64_f, 1.0)
        eps_sb = singles.tile([128, 1], F32)
        nc.vector.memset(eps_sb, EPS)

        wc_sb = singles.tile([128, 4, 4], F32)
        nc.sync.dma_start(wc_sb, wconv_d.rearrange("(cc p) t -> p cc t", p=128))
        bc_sb = singles.tile([128, 4], F32)
        nc.sync.dma_start(bc_sb, bconv_d.rearrange("(cc p) o -> p (cc o)", p=128))
        gam_sb = singles.tile([128, 4], F32)
        nc.sync.dma_start(gam_sb, gam_d.rearrange("(cc p) o -> p (cc o)", p=128))
        bet_sb = singles.tile([128, 4], F32)
        nc.sync.dma_start(bet_sb, bet_d.rearrange("(cc p) o -> p (cc o)", p=128))
        wl_sb = singles.tile([128, 4, 128], F32R)
        nc.sync.dma_start(wl_sb, wlin_d.rearrange("(cc p) j -> p cc j", p=128))
        blin_sb = singles.tile([128, 1], F32)
        nc.sync.dma_start(blin_sb, blin_d)
        wp_sb = singles.tile([128, 2, 512], F32R)
        nc.sync.dma_start(wp_sb, wproj_d.rearrange("(g p) c -> p g c", p=128))

        # whole-kernel big tensors
        qkvT = [main.tile([128, N], F32R, tag=f"qkvT{j}", name=f"qkvT{j}") for j in range(6)]
        # x2 with zero-masked halves: x2m[0] rows 0-63 = x2, rows 64-127 = 0;
        # x2m[1] rows 0-63 = 0, rows 64-127 = x2.  Full-K matmuls then select a head.
        x2m = [main.tile([128, M], F32R, tag=f"x2m{i}", name=f"x2m{i}") for i in range(2)]
        zf = main.tile([128, M], F32, tag="zf")
        nc.vector.memset(zf, 0.0)

        # ================= P1-P3: transpose x, SR branch, qkv projection =================
        with tc.tile_pool(name="early", bufs=1) as early, \
             tc.tile_pool(name="psE", bufs=1, space="PSUM") as psE:

            wq_sb = early.tile([128, 4, 768], F32R, tag="wq")
            nc.sync.dma_start(wq_sb, wqkv_d.rearrange("(cc p) j -> p cc j", p=128))

            y = [early.tile([128, M], F32R, tag=f"ych{cc}", name=f"ych{cc}") for cc in range(4)]
            mu_bb = early.tile([128, M], F32, tag="mu_bb")
            rstd_bb = early.tile([128, M], F32, tag="rstd_bb")

            st_psum = None
            strips = [None, None]
            for nb in range(8):           # 8 strips of 512 tokens
                xnat = [early.tile([128, 512], F32R, tag="xnat", bufs=5, name=f"xnat{nb}_{t_}") for t_ in range(4)]
                for t in range(4):
                    nc.sync.dma_start(xnat[t], x_d[nb * 512 + t * 128: nb * 512 + (t + 1) * 128, :])
                strip = early.tile([128, 4, 512], F32R, tag="strip", bufs=2)
                strips[nb % 2] = strip
                for cc in range(4):
                    tp = psE.tile([128, 512], F32R, tag="tp", bufs=2)
                    for t in range(4):
                        nc.tensor.transpose(tp[:, t * 128:(t + 1) * 128],
                                            xnat[t][:, cc * 128:(cc + 1) * 128], ident)
                    nc.vector.tensor_copy(strip[:, cc, :], tp)

                # conv taps for this strip -> y[:, nb*128:(nb+1)*128]
                for cc in range(4):
                    sv = strip[:, cc, :].rearrange("p (i2 di j dj) -> p i2 di j dj",
                                                   i2=4, di=2, j=32, dj=2)
                    yv = y[cc][:, nb * 128:(nb + 1) * 128].rearrange("p (i2 j) -> p i2 j", i2=4)
                    nc.vector.tensor_scalar(yv, sv[:, :, 0, :, 0],
                                            wc_sb[:, cc, 0:1], bc_sb[:, cc:cc + 1],
                                            OP.mult, OP.add)
                    for t, (di, dj) in enumerate([(0, 1), (1, 0), (1, 1)], start=1):
                        nc.vector.scalar_tensor_tensor(yv, sv[:, :, di, :, dj],
                                                       wc_sb[:, cc, t:t + 1], yv,
                                                       op0=OP.mult, op1=OP.add)

                # qkv projection for this strip
                for jg in range(6):
                    qk = psE.tile([128, 512], F32, tag="qkv", bufs=2)
                    for cc in range(4):
                        nc.tensor.matmul(qk, wq_sb[:, cc, jg * 128:(jg + 1) * 128],
                                         strip[:, cc, :], start=(cc == 0), stop=(cc == 3))
                    nc.scalar.copy(qkvT[jg][:, nb * 512:(nb + 1) * 512], qk)

            # ---- LayerNorm stats over C (partition direction, via ones-matmuls) ----
            st_psum = psE.tile([128, M], F32, tag="p2", bufs=2)
            sq_psum = psE.tile([128, M], F32, tag="p2", bufs=2)
            y2 = [None] * 4
            for cc in range(4):
                y2t = early.tile([128, M], F32R, tag="ysq", bufs=1)
                nc.vector.tensor_mul(y2t, y[cc], y[cc])
                y2[cc] = y2t
            for mh in range(2):
                for cc in range(4):
                    nc.tensor.matmul(st_psum[0:1, mh * 512:(mh + 1) * 512], ones_col_r,
                                     y[cc][:, mh * 512:(mh + 1) * 512],
                                     start=(cc == 0), stop=(cc == 3))
                    nc.tensor.matmul(sq_psum[0:1, mh * 512:(mh + 1) * 512], ones_col_r,
                                     y2[cc][:, mh * 512:(mh + 1) * 512],
                                     start=(cc == 0), stop=(cc == 3))
            mu_sb = early.tile([1, M], F32R, tag="mu")
            nc.vector.tensor_scalar_mul(mu_sb, st_psum[0:1, :], 1.0 / C)
            ms_sb = early.tile([1, M], F32, tag="ms")
            nc.vector.tensor_scalar_mul(ms_sb, sq_psum[0:1, :], 1.0 / C)
            tmp_row = early.tile([1, M], F32, tag="tmp_row")
            nc.vector.tensor_mul(tmp_row, mu_sb, mu_sb)          # mu^2
            nc.vector.tensor_sub(ms_sb, ms_sb, tmp_row)          # var (in place)
            nc.scalar.activation(tmp_row, ms_sb, AF.Sqrt, bias=eps_sb[0:1, :], scale=1.0)
            rstd = early.tile([1, M], F32R, tag="rstd")
            nc.vector.reciprocal(rstd, tmp_row)

            # broadcast mu, rstd across partitions via K=1 matmuls
            bcp = psE.tile([128, M], F32, tag="p2", bufs=2)
            for mh in range(2):
                nc.tensor.matmul(bcp[:, mh * 512:(mh + 1) * 512], ones_row_r,
                                 mu_sb[:, mh * 512:(mh + 1) * 512], start=True, stop=True)
            nc.vector.tensor_copy(mu_bb, bcp)
            bcp2 = psE.tile([128, M], F32, tag="p2", bufs=2)
            for mh in range(2):
                nc.tensor.matmul(bcp2[:, mh * 512:(mh + 1) * 512], ones_row_r,
                                 rstd[:, mh * 512:(mh + 1) * 512], start=True, stop=True)
            nc.scalar.copy(rstd_bb, bcp2)

            # normalize + gamma/beta, then linear to x2 (duplicated rows)
            x2p = psE.tile([128, M], F32, tag="p2", bufs=2)
            for cc in range(4):
                t1 = early.tile([128, M], F32, tag="t1", bufs=2)
                nc.vector.tensor_sub(t1, y[cc], mu_bb)
                nc.vector.scalar_tensor_tensor(t1, t1, gam_sb[:, cc:cc + 1], rstd_bb,
                                               op0=OP.mult, op1=OP.mult)
                nc.scalar.activation(y[cc], t1, AF.Identity,
                                     bias=bet_sb[:, cc:cc + 1], scale=1.0)
            for mh in range(2):
                for cc in range(4):
                    nc.tensor.matmul(x2p[:, mh * 512:(mh + 1) * 512], wl_sb[:, cc, :],
                                     y[cc][:, mh * 512:(mh + 1) * 512],
                                     start=(cc == 0), stop=(cc == 3))
            for i in range(2):
                nc.scalar.activation(x2m[i], x2p, AF.Identity, bias=blin_sb, scale=1.0)
            nc.vector.tensor_copy(x2m[0][64:128, :], zf[64:128, :])
            nc.vector.tensor_copy(x2m[1][0:64, :], zf[0:64, :])

        # ================= P4-P5: token-major k/v, scores + pooling =================
        with tc.tile_pool(name="late", bufs=1) as late:
            psM_ctx = tc.tile_pool(name="psM", bufs=1, space="PSUM")
            psM = psM_ctx.__enter__()

            # token-major k/v with ones column: [128, 32 chunks, 65]
            kc_use = [late.tile([128, M], F32R, tag=f"kcu{h}", name=f"kcu{h}") for h in range(HL)]
            sscale = [late.tile([128, 8], F32, tag=f"ss{h}", name=f"ss{h}") for h in range(HL)]
            vtok = [late.tile([128, 8 * 66 + 62], F32R, tag=f"vt{h}", name=f"vt{h}") for h in range(HL)]

            for h in range(HL):
                base = 64 * (h % 2)
                kT = qkvT[2 + h // 2]
                vT = qkvT[4 + h // 2]
                idn = ident[base:base + 64, base:base + 64]

                ktok = late.tile([128, 32 * 66 + 62], F32R, tag="ktok", bufs=1)
                vtok_full = late.tile([128, 32 * 66 + 62], F32R, tag="vtokf", bufs=1)
                for src, dst in ((kT, ktok), (vT, vtok_full)):
                    dst3 = dst[:, 0:32 * 66].rearrange("p (c w) -> p c w", w=66)
                    for g8 in range(4):
                        tp2 = psM.tile([128, 512], F32R, tag="sc", bufs=2)
                        for t in range(8):
                            ncnk = g8 * 8 + t
                            nc.tensor.transpose(
                                tp2[:, t * 64:(t + 1) * 64],
                                src[base:base + 64, ncnk * 128:(ncnk + 1) * 128], idn)
                        ov = dst3[:, g8 * 8:(g8 + 1) * 8, 0:64]
                        nc.vector.tensor_copy(ov, tp2.rearrange("p (t d) -> p t d", t=8))
                    nc.vector.tensor_copy(
                        dst3[:, :, 64:66],
                        ones64_f.rearrange("p (a o) -> p a o", o=2))

                pool_k = psM.tile([128, M], F32, tag="poolk")
                pool_v = psM.tile([128, M], F32, tag="poolv")
                x2u = x2m[h % 2]
                for ncnk in range(32):
                    for src_tok, pool_ps, featT in ((ktok, pool_k, kT), (vtok_full, pool_v, vT)):
                        sc = psM.tile([128, M], F32, tag="sc", bufs=2)
                        for mh in range(2):
                            nc.tensor.matmul(sc[:, mh * 512:(mh + 1) * 512],
                                             featT[:, ncnk * 128:(ncnk + 1) * 128],
                                             x2u[:, mh * 512:(mh + 1) * 512],
                                             start=True, stop=True)
                        exps = late.tile([128, M], F32R, tag="expS", bufs=2)
                        nc.scalar.activation(exps, sc, AF.Exp)
                        for mh in range(2):
                            nc.tensor.matmul(pool_ps[:, mh * 512:(mh + 1) * 512],
                                             src_tok[:, ncnk * 66: ncnk * 66 + 128],
                                             exps[:, mh * 512:(mh + 1) * 512],
                                             start=(ncnk == 0), stop=(ncnk == 31))

                # evict pools; k_c into parity-aligned, zero-masked kc_use tile
                kc_ev = late.tile([128, M], F32R, tag="kctmp", bufs=1)
                nc.vector.tensor_copy(kc_ev[0:66, :], pool_k[0:66, :])
                if h % 2 == 0:
                    nc.vector.tensor_copy(kc_use[h][0:64, :], kc_ev[0:64, :])
                    nc.vector.tensor_copy(kc_use[h][64:128, :], zf[64:128, :])
                else:
                    nc.sync.dma_start(kc_use[h][64:128, :], kc_ev[0:64, :])
                    nc.vector.tensor_copy(kc_use[h][0:64, :], zf[0:64, :])
                vc_ev = late.tile([128, M], F32R, tag="vcev", bufs=1)
                nc.vector.tensor_copy(vc_ev[0:66, :], pool_v[0:66, :])

                # denominators: transpose-pack Dk|Dv rows -> [128, 16], recip
                dkp = psM.tile([128, 32], F32R, tag="sc", bufs=2)
                for mc in range(8):
                    nc.tensor.transpose(dkp[:, mc * 2:(mc + 1) * 2],
                                        kc_ev[64:66, mc * 128:(mc + 1) * 128],
                                        ident[64:66, 64:66])
                    nc.tensor.transpose(dkp[:, 16 + mc * 2:18 + mc * 2],
                                        vc_ev[64:66, mc * 128:(mc + 1) * 128],
                                        ident[64:66, 64:66])
                dd = late.tile([128, 32], F32, tag="dd", bufs=2)
                nc.vector.tensor_copy(dd, dkp)
                rr = late.tile([128, 32], F32, tag="rr", bufs=2)
                nc.vector.reciprocal(rr, dd)
                nc.vector.tensor_scalar_mul(sscale[h], rr[:, 0:16:2], SCALE)

                # vtok: transpose v_c rows, scale cols 0-63 by 1/Dv, ones col 64
                vtp = psM.tile([128, 528], F32R, tag="sc", bufs=2)
                for mc in range(8):
                    nc.tensor.transpose(vtp[:, mc * 66:(mc + 1) * 66],
                                        vc_ev[0:66, mc * 128:(mc + 1) * 128],
                                        ident[0:66, 0:66])
                vt3 = vtok[h][:, 0:8 * 66].rearrange("p (c w) -> p c w", w=66)
                for mc in range(8):
                    nc.vector.tensor_scalar_mul(vt3[:, mc, 0:64],
                                                vtp[:, mc * 66:mc * 66 + 64],
                                                rr[:, 16 + mc * 2:17 + mc * 2])
                nc.vector.tensor_copy(
                    vt3[:, :, 64:66],
                    ones64_f[:, 0:16].rearrange("p (a o) -> p a o", o=2))

            # ================= P6: attention =================
            psM_ctx.__exit__(None, None, None)
            with tc.tile_pool(name="psL", bufs=1, space="PSUM") as psL:
                for nq in range(4):
                    for h in range(HL):
                        qT = qkvT[h // 2]
                        outp = psL.tile([128, 1024], F32, tag="out", bufs=2)
                        for mc in range(8):
                            lg = psL.tile([128, 1024], F32, tag="lg", bufs=2)
                            for n2 in range(2):
                                nc.tensor.matmul(
                                    lg[:, n2 * 512:(n2 + 1) * 512],
                                    kc_use[h][:, mc * 128:(mc + 1) * 128],
                                    qT[:, nq * 1024 + n2 * 512: nq * 1024 + (n2 + 1) * 512],
                                    start=True, stop=True)
                            expl = late.tile([128, 1024], F32R, tag="expL", bufs=3)
                            nc.scalar.activation(expl, lg, AF.Exp,
                                                 bias=0.0, scale=sscale[h][:, mc:mc + 1])
                            for n2 in range(2):
                                nc.tensor.matmul(outp[:, n2 * 512:(n2 + 1) * 512],
                                                 vtok[h][:, mc * 66: mc * 66 + 128],
                                                 expl[:, n2 * 512:(n2 + 1) * 512],
                                                 start=(mc == 0), stop=(mc == 7))
                        # normalize by DL (row 64)
                        rdl = late.tile([1, 1024], F32R, tag="rdl", bufs=1)
                        nc.vector.reciprocal(rdl, outp[64:65, :])
                        bc2 = late.tile([64, 1024], F32R, tag="bc2", bufs=1)
                        nc.gpsimd.partition_broadcast(bc2, rdl)
                        ost = late.tile([64, 1024], F32R, tag="ost", bufs=1)
                        nc.vector.tensor_mul(ost, outp[0:64, :], bc2)
                        nc.sync.dma_start(oT_d[:, h, nq * 1024:(nq + 1) * 1024], ost)

                    # ---- output projection for this nq's 8 token blocks ----
                    for nb in range(nq * 8, nq * 8 + 8):
                        ld = late.tile([128, 2, 128], F32R, tag="ld", bufs=4)
                        for g in range(2):
                            src = bass.AP(tensor=oT_d.tensor,
                                          offset=(2 * g) * N + nb * 128,
                                          ap=[[N, 2], [HL * N, 64], [1, 128]])
                            nc.sync.dma_start(ld[:, g, :].rearrange("p (a n) -> p a n", a=1), src)
                        pj = psL.tile([128, 512], F32, tag="lg", bufs=2)
                        for g in range(2):
                            nc.tensor.matmul(pj, ld[:, g, :], wp_sb[:, g, :],
                                             start=(g == 0), stop=(g == 1))
                        o_sb = late.tile([128, 512], F32, tag="osb", bufs=2)
                        if nb % 2 == 0:
                            nc.vector.tensor_copy(o_sb, pj)
                        else:
                            nc.scalar.copy(o_sb, pj)
                        nc.sync.dma_start(out_d[nb * 128:(nb + 1) * 128, :], o_sb)

    nc.compile()
    return nc


def _get_nc():
    if "nc" not in _CACHE:
        _CACHE["nc"] = _build_nc()
    return _CACHE["nc"]


def _make_in_maps(x, w_qkv, w_proj, w_sr_conv, b_sr_conv,
                  sr_norm_scale, sr_norm_bias, w_sr_linear, b_sr_linear):
    f32 = np.float32
    x = np.ascontiguousarray(np.asarray(x, f32))
    wq4 = np.asarray(w_qkv, f32).reshape(C, 3, H, HD)
    wp3 = np.asarray(w_proj, f32).reshape(H, HD, C)
    wconv = np.ascontiguousarray(np.asarray(w_sr_conv, f32).reshape(C, 4))
    bconv = np.asarray(b_sr_conv, f32).reshape(C, 1)
    gam = np.asarray(sr_norm_scale, f32).reshape(C, 1)
    bet = np.asarray(sr_norm_bias, f32).reshape(C, 1)
    wlin = np.asarray(w_sr_linear, f32)                    # [C, 64]
    wlin2 = np.ascontiguousarray(np.concatenate([wlin, wlin], axis=1))   # [C, 128]
    blin = np.asarray(b_sr_linear, f32).reshape(HD, 1)
    blin2 = np.ascontiguousarray(np.concatenate([blin, blin], axis=0))   # [128, 1]

    in_maps = []
    for core in range(8):
        b = core // 2
        g = core % 2
        hs = slice(g * HL, (g + 1) * HL)
        wq_slice = np.ascontiguousarray(wq4[:, :, hs, :].reshape(C, 3 * HL * HD))
        wp_slice = np.ascontiguousarray(wp3[hs].reshape(HL * HD, C))
        in_maps.append(dict(
            x=np.ascontiguousarray(x[b]),
            wqkv=wq_slice,
            wconv=wconv,
            bconv=bconv,
            gam=gam,
            bet=bet,
            wlin=wlin2,
            blin=blin2,
            wproj=wp_slice,
        ))
    return in_maps


def kernel(x, w_qkv, w_proj, b_proj, w_sr_conv, b_sr_conv,
           sr_norm_scale, sr_norm_bias, w_sr_linear, b_sr_linear,
           _trace=False, _trace_kwargs=None):
    nc = _get_nc()
    in_maps = _make_in_maps(x, w_qkv, w_proj, w_sr_conv, b_sr_conv,
                            sr_norm_scale, sr_norm_bias, w_sr_linear, b_sr_linear)
    res = bass_utils.run_bass_kernel_spmd(
        nc, in_maps, list(range(8)), trace=_trace, **(_trace_kwargs or {}))
    bp = np.asarray(b_proj, np.float32)
    out = np.empty((B, N, C), np.float32)
    for b in range(B):
        out[b] = res.results[2 * b]["out"] + res.results[2 * b + 1]["out"] + bp
    if _trace:
        return out, res
    return out


# revision 26
# speedup vs baseline: 1.0643x; 1.0643x over previous
"""Trainium2 Bass kernel for nn_Attention3D (spatial-reduction attention).

Sharding: 8 cores = 4 batches x 2 head-groups (4 heads each).
Each core computes, for its (batch b, heads 4g..4g+3):
  qkv = x_b @ Wqkv_slice            (feature-major qkvT layout, fp32r matmuls)
  SR branch: depthwise 2x2/2 conv -> LayerNorm -> linear  => x2 [64, 1024]
  scores_k/v = softmax_n(x2 . k/v)  (exp on ACT; denominators via ones-column)
  k_c/v_c    = softmax-weighted pooling of k/v
  attn       = softmax_m(q . k_c * scale); out = attn . v_c
  partial    = out_heads @ Wproj_rows    (host sums the two head-group partials)

Host-side: slice inputs per core, run SPMD on 8 NeuronCores, sum partials + b_proj.
"""
import numpy as np
from contextlib import ExitStack

import concourse.bass as bass
import concourse.tile as tile
from concourse import bacc, mybir
from concourse import bass_utils
from concourse.masks import make_identity
from concourse.alu_op_type import AluOpType

# problem constants
B, N, C, H, HD, SR = 4, 4096, 512, 8, 64, 2
M = 1024              # pooled tokens: (64/2)^2
HL = 4                # heads per core
SCALE = HD ** -0.5
EPS = 1e-5

F32 = mybir.dt.float32
F32R = mybir.dt.float32r
AF = mybir.ActivationFunctionType
OP = AluOpType

_CACHE = {}


def _build_nc():
    nc = bacc.Bacc("TRN2", target_bir_lowering=False, debug=False, num_devices=8)

    # ---- DRAM I/O (per-core shard shapes) ----
    x_d = nc.dram_tensor("x", [N, C], F32R, kind="ExternalInput").ap()
    wqkv_d = nc.dram_tensor("wqkv", [C, 3 * HL * HD], F32R, kind="ExternalInput").ap()
    wconv_d = nc.dram_tensor("wconv", [C, 4], F32, kind="ExternalInput").ap()
    bconv_d = nc.dram_tensor("bconv", [C, 1], F32, kind="ExternalInput").ap()
    gam_d = nc.dram_tensor("gam", [C, 1], F32, kind="ExternalInput").ap()
    bet_d = nc.dram_tensor("bet", [C, 1], F32, kind="ExternalInput").ap()
    wlin_d = nc.dram_tensor("wlin", [C, 128], F32R, kind="ExternalInput").ap()   # duplicated cols
    blin_d = nc.dram_tensor("blin", [128, 1], F32, kind="ExternalInput").ap()    # duplicated rows
    wproj_d = nc.dram_tensor("wproj", [HL * HD, C], F32R, kind="ExternalInput").ap()
    out_d = nc.dram_tensor("out", [N, C], F32, kind="ExternalOutput").ap()
    # scratch for normalized per-head attention outputs, feature-major [d, h, n]
    oT_d = nc.dram_tensor("oT", [HD, HL, N], F32R, kind="Internal").ap()

    with tile.TileContext(nc) as tc, \
         nc.allow_low_precision("fp32r operands are intentional"), \
         ExitStack() as ctx:
        # ---------------- persistent pools ----------------
        singles = ctx.enter_context(tc.tile_pool(name="singles", bufs=1))
        main = ctx.enter_context(tc.tile_pool(name="main", bufs=1))

        ident_f = singles.tile([128, 128], F32)
        make_identity(nc, ident_f)
        ident = singles.tile([128, 128], F32R)
        nc.vector.tensor_copy(ident, ident_f)

        ones_col_f = singles.tile([128, 1], F32)
        nc.vector.memset(ones_col_f, 1.0)
        ones_col_r = singles.tile([128, 1], F32R)
        nc.vector.tensor_copy(ones_col_r, ones_col_f)
        ones_row_r = singles.tile([1, 128], F32R)
        nc.vector.tensor_copy(ones_row_r, ident_f[0:1, :])  # placeholder; fixed below
        ones_row_f = singles.tile([1, 128], F32)
        nc.vector.memset(ones_row_f, 1.0)
        nc.vector.tensor_copy(ones_row_r, ones_row_f)
        ones32_f = singles.tile([128, 32], F32)
        nc.vector.memset(ones32_f, 1.0)
        ones64_f = singles.tile([128, 64], F32)
        nc.vector.memset(ones64_f, 1.0)
        eps_sb = singles.tile([128, 1], F32)
        nc.vector.memset(eps_sb, EPS)

        wc_sb = singles.tile([128, 4, 4], F32)
        nc.sync.dma_start(wc_sb, wconv_d.rearrange("(cc p) t -> p cc t", p=128))
        bc_sb = singles.tile([128, 4], F32)
        nc.sync.dma_start(bc_sb, bconv_d.rearrange("(cc p) o -> p (cc o)", p=128))
        gam_sb = singles.tile([128, 4], F32)
        nc.sync.dma_start(gam_sb, gam_d.rearrange("(cc p) o -> p (cc o)", p=128))
        bet_sb = singles.tile([128, 4], F32)
        nc.sync.dma_start(bet_sb, bet_d.rearrange("(cc p) o -> p (cc o)", p=128))
        wl_sb = singles.tile([128, 4, 128], F32R)
        nc.sync.dma_start(wl_sb, wlin_d.rearrange("(cc p) j -> p cc j", p=128))
        blin_sb = singles.tile([128, 1], F32)
        nc.sync.dma_start(blin_sb, blin_d)
        wp_sb = singles.tile([128, 2, 512], F32R)
        nc.sync.dma_start(wp_sb, wproj_d.rearrange("(g p) c -> p g c", p=128))

        # whole-kernel big tensors
        qkvT = [main.tile([128, N], F32R, tag=f"qkvT{j}", name=f"qkvT{j}") for j in range(6)]
        # x2 with zero-masked halves: x2m[0] rows 0-63 = x2, rows 64-127 = 0;
        # x2m[1] rows 0-63 = 0, rows 64-127 = x2.  Full-K matmuls then select a head.
        x2m = [main.tile([128, M], F32R, tag=f"x2m{i}", name=f"x2m{i}") for i in range(2)]
        zf = main.tile([128, M], F32, tag="zf")
        nc.vector.memset(zf, 0.0)

        # ================= P1-P3: transpose x, SR branch, qkv projection =================
        with tc.tile_pool(name="early", bufs=1) as early, \
             tc.tile_pool(name="psE", bufs=1, space="PSUM") as psE:

            wq_sb = early.tile([128, 4, 768], F32R, tag="wq")
            nc.sync.dma_start(wq_sb, wqkv_d.rearrange("(cc p) j -> p cc j", p=128))

            y = [early.tile([128, M], F32R, tag=f"ych{cc}", name=f"ych{cc}") for cc in range(4)]
            mu_bb = early.tile([128, M], F32, tag="mu_bb")
            rstd_bb = early.tile([128, M], F32, tag="rstd_bb")

            st_psum = None
            strips = [None, None]
            for nb in range(8):           # 8 strips of 512 tokens
                xnat = [early.tile([128, 512], F32R, tag="xnat", bufs=5, name=f"xnat{nb}_{t_}") for t_ in range(4)]
                for t in range(4):
                    nc.sync.dma_start(xnat[t], x_d[nb * 512 + t * 128: nb * 512 + (t + 1) * 128, :])
                strip = early.tile([128, 4, 512], F32R, tag="strip", bufs=2)
                strips[nb % 2] = strip
                for cc in range(4):
                    tp = psE.tile([128, 512], F32R, tag="tp", bufs=2)
                    for t in range(4):
                        nc.tensor.transpose(tp[:, t * 128:(t + 1) * 128],
                                            xnat[t][:, cc * 128:(cc + 1) * 128], ident)
                    nc.vector.tensor_copy(strip[:, cc, :], tp)

                # conv taps for this strip -> y[:, nb*128:(nb+1)*128]
                for cc in range(4):
                    sv = strip[:, cc, :].rearrange("p (i2 di j dj) -> p i2 di j dj",
                                                   i2=4, di=2, j=32, dj=2)
                    yv = y[cc][:, nb * 128:(nb + 1) * 128].rearrange("p (i2 j) -> p i2 j", i2=4)
                    nc.vector.tensor_scalar(yv, sv[:, :, 0, :, 0],
                                            wc_sb[:, cc, 0:1], bc_sb[:, cc:cc + 1],
                                            OP.mult, OP.add)
                    for t, (di, dj) in enumerate([(0, 1), (1, 0), (1, 1)], start=1):
                        nc.vector.scalar_tensor_tensor(yv, sv[:, :, di, :, dj],
                                                       wc_sb[:, cc, t:t + 1], yv,
                                                       op0=OP.mult, op1=OP.add)

                # qkv projection for this strip
                for jg in range(6):
                    qk = psE.tile([128, 512], F32, tag="qkv", bufs=2)
                    for cc in range(4):
                        nc.tensor.matmul(qk, wq_sb[:, cc, jg * 128:(jg + 1) * 128],
                                         strip[:, cc, :], start=(cc == 0), stop=(cc == 3))
                    nc.scalar.copy(qkvT[jg][:, nb * 512:(nb + 1) * 512], qk)

            # ---- LayerNorm stats over C (partition direction, via ones-matmuls) ----
            st_psum = psE.tile([128, M], F32, tag="p2", bufs=2)
            sq_psum = psE.tile([128, M], F32, tag="p2", bufs=2)
            y2 = [None] * 4
            for cc in range(4):
                y2t = early.tile([128, M], F32R, tag="ysq", bufs=1)
                nc.vector.tensor_mul(y2t, y[cc], y[cc])
                y2[cc] = y2t
            for mh in range(2):
                for cc in range(4):
                    nc.tensor.matmul(st_psum[0:1, mh * 512:(mh + 1) * 512], ones_col_r,
                                     y[cc][:, mh * 512:(mh + 1) * 512],
                                     start=(cc == 0), stop=(cc == 3))
                    nc.tensor.matmul(sq_psum[0:1, mh * 512:(mh + 1) * 512], ones_col_r,
                                     y2[cc][:, mh * 512:(mh + 1) * 512],
                                     start=(cc == 0), stop=(cc == 3))
            mu_sb = early.tile([1, M], F32R, tag="mu")
            nc.vector.tensor_scalar_mul(mu_sb, st_psum[0:1, :], 1.0 / C)
            ms_sb = early.tile([1, M], F32, tag="ms")
            nc.vector.tensor_scalar_mul(ms_sb, sq_psum[0:1, :], 1.0 / C)
            tmp_row = early.tile([1, M], F32, tag="tmp_row")
            nc.vector.tensor_mul(tmp_row, mu_sb, mu_sb)          # mu^2
            nc.vector.tensor_sub(ms_sb, ms_sb, tmp_row)          # var (in place)
            nc.scalar.activation(tmp_row, ms_sb, AF.Sqrt, bias=eps_sb[0:1, :], scale=1.0)
            rstd = early.tile([1, M], F32R, tag="rstd")
            nc.vector.reciprocal(rstd, tmp_row)

            # broadcast mu, rstd across partitions via K=1 matmuls
            bcp = psE.tile([128, M], F32, tag="p2", bufs=2)
            for mh in range(2):
                nc.tensor.matmul(bcp[:, mh * 512:(mh + 1) * 512], ones_row_r,
                                 mu_sb[:, mh * 512:(mh + 1) * 512], start=True, stop=True)
            nc.vector.tensor_copy(mu_bb, bcp)
            bcp2 = psE.tile([128, M], F32, tag="p2", bufs=2)
            for mh in range(2):
                nc.tensor.matmul(bcp2[:, mh * 512:(mh + 1) * 512], ones_row_r,
                                 rstd[:, mh * 512:(mh + 1) * 512], start=True, stop=True)
            nc.scalar.copy(rstd_bb, bcp2)

            # normalize + gamma/beta, then linear to x2 (duplicated rows)
            x2p = psE.tile([128, M], F32, tag="p2", bufs=2)
            for cc in range(4):
                t1 = early.tile([128, M], F32, tag="t1", bufs=2)
                nc.vector.tensor_sub(t1, y[cc], mu_bb)
                nc.vector.scalar_tensor_tensor(t1, t1, gam_sb[:, cc:cc + 1], rstd_bb,
                                               op0=OP.mult, op1=OP.mult)
                nc.scalar.activation(y[cc], t1, AF.Identity,
                                     bias=bet_sb[:, cc:cc + 1], scale=1.0)
            for mh in range(2):
                for cc in range(4):
                    nc.tensor.matmul(x2p[:, mh * 512:(mh + 1) * 512], wl_sb[:, cc, :],
                                     y[cc][:, mh * 512:(mh + 1) * 512],
                                     start=(cc == 0), stop=(cc == 3))
            for i in range(2):
                nc.scalar.activation(x2m[i], x2p, AF.Identity, bias=blin_sb, scale=1.0)
            nc.vector.tensor_copy(x2m[0][64:128, :], zf[64:128, :])
            nc.vector.tensor_copy(x2m[1][0:64, :], zf[0:64, :])

        # ================= P4-P5: token-major k/v, scores + pooling =================
        with tc.tile_pool(name="late", bufs=1) as late:
            psM_ctx = tc.tile_pool(name="psM", bufs=1, space="PSUM")
            psM = psM_ctx.__enter__()

            # token-major k/v with ones column: [128, 32 chunks, 65]
            kc_use = [late.tile([128, M], F32R, tag=f"kcu{h}", name=f"kcu{h}") for h in range(HL)]
            sscale = [late.tile([128, 8], F32, tag=f"ss{h}", name=f"ss{h}") for h in range(HL)]
            vtok = [late.tile([128, 8 * 66 + 62], F32R, tag=f"vt{h}", name=f"vt{h}") for h in range(HL)]

            for h in range(HL):
                base = 64 * (h % 2)
                kT = qkvT[2 + h // 2]
                vT = qkvT[4 + h // 2]
                idn = ident[base:base + 64, base:base + 64]

                ktok = late.tile([128, 32 * 66 + 62], F32R, tag="ktok", bufs=1)
                vtok_full = late.tile([128, 32 * 66 + 62], F32R, tag="vtokf", bufs=1)
                for src, dst in ((kT, ktok), (vT, vtok_full)):
                    dst3 = dst[:, 0:32 * 66].rearrange("p (c w) -> p c w", w=66)
                    for g8 in range(4):
                        tp2 = psM.tile([128, 512], F32R, tag="sc", bufs=2)
                        for t in range(8):
                            ncnk = g8 * 8 + t
                            nc.tensor.transpose(
                                tp2[:, t * 64:(t + 1) * 64],
                                src[base:base + 64, ncnk * 128:(ncnk + 1) * 128], idn)
                        ov = dst3[:, g8 * 8:(g8 + 1) * 8, 0:64]
                        nc.vector.tensor_copy(ov, tp2.rearrange("p (t d) -> p t d", t=8))
                    nc.vector.tensor_copy(
                        dst3[:, :, 64:66],
                        ones64_f.rearrange("p (a o) -> p a o", o=2))

                pool_k = psM.tile([128, M], F32, tag="poolk")
                pool_v = psM.tile([128, M], F32, tag="poolv")
                x2u = x2m[h % 2]
                for ncnk in range(32):
                    for src_tok, pool_ps, featT in ((ktok, pool_k, kT), (vtok_full, pool_v, vT)):
                        sc = psM.tile([128, M], F32, tag="sc", bufs=2)
                        for mh in range(2):
                            nc.tensor.matmul(sc[:, mh * 512:(mh + 1) * 512],
                                             featT[:, ncnk * 128:(ncnk + 1) * 128],
                                             x2u[:, mh * 512:(mh + 1) * 512],
                                             start=True, stop=True)
                        exps = late.tile([128, M], F32R, tag="expS", bufs=2)
                        nc.scalar.activation(exps, sc, AF.Exp)
                        for mh in range(2):
                            nc.tensor.matmul(pool_ps[:, mh * 512:(mh + 1) * 512],
                                             src_tok[:, ncnk * 66: ncnk * 66 + 128],
                                             exps[:, mh * 512:(mh + 1) * 512],
                                             start=(ncnk == 0), stop=(ncnk == 31))

                # evict pools; k_c into parity-aligned, zero-masked kc_use tile
                kc_ev = late.tile([128, M], F32R, tag="kctmp", bufs=1)
                nc.vector.tensor_copy(kc_ev[0:66, :], pool_k[0:66, :])
                if h % 2 == 0:
                    nc.vector.tensor_copy(kc_use[h][0:64, :], kc_ev[0:64, :])
                    nc.vector.tensor_copy(kc_use[h][64:128, :], zf[64:128, :])
                else:
                    nc.sync.dma_start(kc_use[h][64:128, :], kc_ev[0:64, :])
                    nc.vector.tensor_copy(kc_use[h][0:64, :], zf[0:64, :])
                vc_ev = late.tile([128, M], F32R, tag="vcev", bufs=1)
                nc.vector.tensor_copy(vc_ev[0:66, :], pool_v[0:66, :])

                # denominators: transpose-pack Dk|Dv rows -> [128, 16], recip
                dkp = psM.tile([128, 32], F32R, tag="sc", bufs=2)
                for mc in range(8):
                    nc.tensor.transpose(dkp[:, mc * 2:(mc + 1) * 2],
                                        kc_ev[64:66, mc * 128:(mc + 1) * 128],
                                        ident[64:66, 64:66])
                    nc.tensor.transpose(dkp[:, 16 + mc * 2:18 + mc * 2],
                                        vc_ev[64:66, mc * 128:(mc + 1) * 128],
                                        ident[64:66, 64:66])
                dd = late.tile([128, 32], F32, tag="dd", bufs=2)
                nc.vector.tensor_copy(dd, dkp)
                rr = late.tile([128, 32], F32, tag="rr", bufs=2)
                nc.vector.reciprocal(rr, dd)
                nc.vector.tensor_scalar_mul(sscale[h], rr[:, 0:16:2], SCALE)

                # vtok: transpose v_c rows, scale cols 0-63 by 1/Dv, ones col 64
                vtp = psM.tile([128, 528], F32R, tag="sc", bufs=2)
                for mc in range(8):
                    nc.tensor.transpose(vtp[:, mc * 66:(mc + 1) * 66],
                                        vc_ev[0:66, mc * 128:(mc + 1) * 128],
                                        ident[0:66, 0:66])
                vt3 = vtok[h][:, 0:8 * 66].rearrange("p (c w) -> p c w", w=66)
                for mc in range(8):
                    nc.vector.tensor_scalar_mul(vt3[:, mc, 0:64],
                                                vtp[:, mc * 66:mc * 66 + 64],
                                                rr[:, 16 + mc * 2:17 + mc * 2])
                nc.vector.tensor_copy(
                    vt3[:, :, 64:66],
                    ones64_f[:, 0:16].rearrange("p (a o) -> p a o", o=2))

            # ================= P6: attention =================
            psM_ctx.__exit__(None, None, None)
            with tc.tile_pool(name="psL", bufs=1, space="PSUM") as psL:
                for nq in range(4):
                    for h in range(HL):
                        qT = qkvT[h // 2]
                        outp = psL.tile([128, 1024], F32, tag="out", bufs=2)
                        for mc in range(8):
                            lg = psL.tile([128, 1024], F32, tag="lg", bufs=2)
                            for n2 in range(2):
                                nc.tensor.matmul(
                                    lg[:, n2 * 512:(n2 + 1) * 512],
                                    kc_use[h][:, mc * 128:(mc + 1) * 128],
                                    qT[:, nq * 1024 + n2 * 512: nq * 1024 + (n2 + 1) * 512],
                                    start=True, stop=True)
                            expl = late.tile([128, 1024], F32R, tag="expL", bufs=2)
                            nc.scalar.activation(expl, lg, AF.Exp,
                                                 bias=0.0, scale=sscale[h][:, mc:mc + 1])
                            for n2 in range(2):
                                nc.tensor.matmul(outp[:, n2 * 512:(n2 + 1) * 512],
                                                 vtok[h][:, mc * 66: mc * 66 + 128],
                                                 expl[:, n2 * 512:(n2 + 1) * 512],
                                                 start=(mc == 0), stop=(mc == 7))
                        # normalize by DL (row 64)
                        rdl = late.tile([1, 1024], F32R, tag="rdl", bufs=1)
                        nc.vector.reciprocal(rdl, outp[64:65, :])
                        bc2 = late.tile([64, 1024], F32R, tag="bc2", bufs=1)
                        nc.gpsimd.partition_broadcast(bc2, rdl)
                        ost = late.tile([64, 1024], F32R, tag="ost", bufs=2)
                        nc.vector.tensor_mul(ost, outp[0:64, :], bc2)
                        nc.sync.dma_start(oT_d[:, h, nq * 1024:(nq + 1) * 1024], ost)

                # ---- output projection (overlaps the attention tail) ----
                for nb in range(32):
                    ld = late.tile([128, 2, 128], F32R, tag="ld", bufs=4)
                    for g in range(2):
                        src = bass.AP(tensor=oT_d.tensor,
                                      offset=(2 * g) * N + nb * 128,
                                      ap=[[N, 2], [HL * N, 64], [1, 128]])
                        nc.sync.dma_start(ld[:, g, :].rearrange("p (a n) -> p a n", a=1), src)
                    pj = psL.tile([128, 512], F32, tag="lg", bufs=2)
                    for g in range(2):
                        nc.tensor.matmul(pj, ld[:, g, :], wp_sb[:, g, :],
                                         start=(g == 0), stop=(g == 1))
                    o_sb = late.tile([128, 512], F32, tag="osb", bufs=2)
                    if nb % 2 == 0:
                        nc.vector.tensor_copy(o_sb, pj)
                    else:
                        nc.scalar.copy(o_sb, pj)
                    nc.sync.dma_start(out_d[nb * 128:(nb + 1) * 128, :], o_sb)

    nc.compile()
    return nc


def _get_nc():
    if "nc" not in _CACHE:
        _CACHE["nc"] = _build_nc()
    return _CACHE["nc"]


def _make_in_maps(x, w_qkv, w_proj, w_sr_conv, b_sr_conv,
                  sr_norm_scale, sr_norm_bias, w_sr_linear, b_sr_linear):
    f32 = np.float32
    x = np.ascontiguousarray(np.asarray(x, f32))
    wq4 = np.asarray(w_qkv, f32).reshape(C, 3, H, HD)
    wp3 = np.asarray(w_proj, f32).reshape(H, HD, C)
    wconv = np.ascontiguousarray(np.asarray(w_sr_conv, f32).reshape(C, 4))
    bconv = np.asarray(b_sr_conv, f32).reshape(C, 1)
    gam = np.asarray(sr_norm_scale, f32).reshape(C, 1)
    bet = np.asarray(sr_norm_bias, f32).reshape(C, 1)
    wlin = np.asarray(w_sr_linear, f32)                    # [C, 64]
    wlin2 = np.ascontiguousarray(np.concatenate([wlin, wlin], axis=1))   # [C, 128]
    blin = np.asarray(b_sr_linear, f32).reshape(HD, 1)
    blin2 = np.ascontiguousarray(np.concatenate([blin, blin], axis=0))   # [128, 1]

    in_maps = []
    for core in range(8):
        b = core // 2
        g = core % 2
        hs = slice(g * HL, (g + 1) * HL)
        wq_slice = np.ascontiguousarray(wq4[:, :, hs, :].reshape(C, 3 * HL * HD))
        wp_slice = np.ascontiguousarray(wp3[hs].reshape(HL * HD, C))
        in_maps.append(dict(
            x=np.ascontiguousarray(x[b]),
            wqkv=wq_slice,
            wconv=wconv,
            bconv=bconv,
            gam=gam,
            bet=bet,
            wlin=wlin2,
            blin=blin2,
            wproj=wp_slice,
        ))
    return in_maps


def kernel(x, w_qkv, w_proj, b_proj, w_sr_conv, b_sr_conv,
           sr_norm_scale, sr_norm_bias, w_sr_linear, b_sr_linear,
           _trace=False, _trace_kwargs=None):
    nc = _get_nc()
    in_maps = _make_in_maps(x, w_qkv, w_proj, w_sr_conv, b_sr_conv,
                            sr_norm_scale, sr_norm_bias, w_sr_linear, b_sr_linear)
    res = bass_utils.run_bass_kernel_spmd(
        nc, in_maps, list(range(8)), trace=_trace, **(_trace_kwargs or {}))
    bp = np.asarray(b_proj, np.float32)
    out = np.empty((B, N, C), np.float32)
    for b in range(B):
        out[b] = res.results[2 * b]["out"] + res.results[2 * b + 1]["out"] + bp
    if _trace:
        return out, res
    return out


# revision 27
# speedup vs baseline: 1.1908x; 1.1189x over previous
"""Trainium2 Bass kernel for nn_Attention3D (spatial-reduction attention).

Sharding: 8 cores = 4 batches x 2 head-groups (4 heads each).
Each core computes, for its (batch b, heads 4g..4g+3):
  qkv = x_b @ Wqkv_slice            (feature-major qkvT layout, fp32r matmuls)
  SR branch: depthwise 2x2/2 conv -> LayerNorm -> linear  => x2 [64, 1024]
  scores_k/v = softmax_n(x2 . k/v)  (exp on ACT; denominators via ones-column)
  k_c/v_c    = softmax-weighted pooling of k/v
  attn       = softmax_m(q . k_c * scale); out = attn . v_c
  partial    = out_heads @ Wproj_rows    (host sums the two head-group partials)

Host-side: slice inputs per core, run SPMD on 8 NeuronCores, sum partials + b_proj.
"""
import numpy as np
from contextlib import ExitStack

import concourse.bass as bass
import concourse.tile as tile
from concourse import bacc, mybir
from concourse import bass_utils
from concourse.masks import make_identity
from concourse.alu_op_type import AluOpType

# problem constants
B, N, C, H, HD, SR = 4, 4096, 512, 8, 64, 2
M = 1024              # pooled tokens: (64/2)^2
HL = 4                # heads per core
SCALE = HD ** -0.5
EPS = 1e-5

F32 = mybir.dt.float32
F32R = mybir.dt.float32r
AF = mybir.ActivationFunctionType
OP = AluOpType

_CACHE = {}


def _build_nc():
    nc = bacc.Bacc("TRN2", target_bir_lowering=False, debug=False, num_devices=8)

    # ---- DRAM I/O (per-core shard shapes) ----
    x_d = nc.dram_tensor("x", [N, C], F32R, kind="ExternalInput").ap()
    wqkv_d = nc.dram_tensor("wqkv", [C, 3 * HL * HD], F32R, kind="ExternalInput").ap()
    wconv_d = nc.dram_tensor("wconv", [C, 4], F32, kind="ExternalInput").ap()
    bconv_d = nc.dram_tensor("bconv", [C, 1], F32, kind="ExternalInput").ap()
    gam_d = nc.dram_tensor("gam", [C, 1], F32, kind="ExternalInput").ap()
    bet_d = nc.dram_tensor("bet", [C, 1], F32, kind="ExternalInput").ap()
    wlin_d = nc.dram_tensor("wlin", [C, 128], F32R, kind="ExternalInput").ap()   # duplicated cols
    blin_d = nc.dram_tensor("blin", [128, 1], F32, kind="ExternalInput").ap()    # duplicated rows
    wproj_d = nc.dram_tensor("wproj", [HL * HD, C], F32R, kind="ExternalInput").ap()
    out_d = nc.dram_tensor("out", [N, C], F32, kind="ExternalOutput").ap()
    # scratch for normalized per-head attention outputs, feature-major [d, h, n]
    oT_d = nc.dram_tensor("oT", [HD, HL, N], F32R, kind="Internal").ap()

    with tile.TileContext(nc) as tc, \
         nc.allow_low_precision("fp32r operands are intentional"), \
         ExitStack() as ctx:
        # ---------------- persistent pools ----------------
        singles = ctx.enter_context(tc.tile_pool(name="singles", bufs=1))
        main = ctx.enter_context(tc.tile_pool(name="main", bufs=1))

        ident_f = singles.tile([128, 128], F32)
        make_identity(nc, ident_f)
        ident = singles.tile([128, 128], F32R)
        nc.vector.tensor_copy(ident, ident_f)

        ones_col_f = singles.tile([128, 1], F32)
        nc.vector.memset(ones_col_f, 1.0)
        ones_col_r = singles.tile([128, 1], F32R)
        nc.vector.tensor_copy(ones_col_r, ones_col_f)
        ones_row_r = singles.tile([1, 128], F32R)
        nc.vector.tensor_copy(ones_row_r, ident_f[0:1, :])  # placeholder; fixed below
        ones_row_f = singles.tile([1, 128], F32)
        nc.vector.memset(ones_row_f, 1.0)
        nc.vector.tensor_copy(ones_row_r, ones_row_f)
        ones32_f = singles.tile([128, 32], F32)
        nc.vector.memset(ones32_f, 1.0)
        ones64_f = singles.tile([128, 64], F32)
        nc.vector.memset(ones64_f, 1.0)
        eps_sb = singles.tile([128, 1], F32)
        nc.vector.memset(eps_sb, EPS)

        wc_sb = singles.tile([128, 4, 4], F32)
        nc.sync.dma_start(wc_sb, wconv_d.rearrange("(cc p) t -> p cc t", p=128))
        bc_sb = singles.tile([128, 4], F32)
        nc.sync.dma_start(bc_sb, bconv_d.rearrange("(cc p) o -> p (cc o)", p=128))
        gam_sb = singles.tile([128, 4], F32)
        nc.sync.dma_start(gam_sb, gam_d.rearrange("(cc p) o -> p (cc o)", p=128))
        bet_sb = singles.tile([128, 4], F32)
        nc.sync.dma_start(bet_sb, bet_d.rearrange("(cc p) o -> p (cc o)", p=128))
        wl_sb = singles.tile([128, 4, 128], F32R)
        nc.sync.dma_start(wl_sb, wlin_d.rearrange("(cc p) j -> p cc j", p=128))
        blin_sb = singles.tile([128, 1], F32)
        nc.sync.dma_start(blin_sb, blin_d)
        wp_sb = singles.tile([128, 2, 512], F32R)
        nc.sync.dma_start(wp_sb, wproj_d.rearrange("(g p) c -> p g c", p=128))

        # whole-kernel big tensors
        qkvT = [main.tile([128, N], F32R, tag=f"qkvT{j}", name=f"qkvT{j}") for j in range(6)]
        # x2 with zero-masked halves: x2m[0] rows 0-63 = x2, rows 64-127 = 0;
        # x2m[1] rows 0-63 = 0, rows 64-127 = x2.  Full-K matmuls then select a head.
        x2m = [main.tile([128, M], F32R, tag=f"x2m{i}", name=f"x2m{i}") for i in range(2)]
        zf = main.tile([128, M], F32, tag="zf")
        nc.vector.memset(zf, 0.0)

        # ================= P1-P3: transpose x, SR branch, qkv projection =================
        with tc.tile_pool(name="early", bufs=1) as early, \
             tc.tile_pool(name="psE", bufs=1, space="PSUM") as psE:

            wq_sb = early.tile([128, 4, 768], F32R, tag="wq")
            nc.sync.dma_start(wq_sb, wqkv_d.rearrange("(cc p) j -> p cc j", p=128))

            y = [early.tile([128, M], F32R, tag=f"ych{cc}", name=f"ych{cc}") for cc in range(4)]
            mu_bb = early.tile([128, M], F32, tag="mu_bb")
            rstd_bb = early.tile([128, M], F32, tag="rstd_bb")

            st_psum = None
            strips = [None, None]
            for nb in range(8):           # 8 strips of 512 tokens
                xnat = [early.tile([128, 512], F32R, tag="xnat", bufs=5, name=f"xnat{nb}_{t_}") for t_ in range(4)]
                for t in range(4):
                    nc.sync.dma_start(xnat[t], x_d[nb * 512 + t * 128: nb * 512 + (t + 1) * 128, :])
                strip = early.tile([128, 4, 512], F32R, tag="strip", bufs=2)
                strips[nb % 2] = strip
                for cc in range(4):
                    tp = psE.tile([128, 512], F32R, tag="tp", bufs=2)
                    for t in range(4):
                        nc.tensor.transpose(tp[:, t * 128:(t + 1) * 128],
                                            xnat[t][:, cc * 128:(cc + 1) * 128], ident)
                    nc.vector.tensor_copy(strip[:, cc, :], tp)

                # conv taps for this strip -> y[:, nb*128:(nb+1)*128]
                for cc in range(4):
                    sv = strip[:, cc, :].rearrange("p (i2 di j dj) -> p i2 di j dj",
                                                   i2=4, di=2, j=32, dj=2)
                    yv = y[cc][:, nb * 128:(nb + 1) * 128].rearrange("p (i2 j) -> p i2 j", i2=4)
                    nc.vector.tensor_scalar(yv, sv[:, :, 0, :, 0],
                                            wc_sb[:, cc, 0:1], bc_sb[:, cc:cc + 1],
                                            OP.mult, OP.add)
                    for t, (di, dj) in enumerate([(0, 1), (1, 0), (1, 1)], start=1):
                        nc.vector.scalar_tensor_tensor(yv, sv[:, :, di, :, dj],
                                                       wc_sb[:, cc, t:t + 1], yv,
                                                       op0=OP.mult, op1=OP.add)

                # qkv projection for this strip
                for jg in range(6):
                    qk = psE.tile([128, 512], F32, tag="qkv", bufs=2)
                    for cc in range(4):
                        nc.tensor.matmul(qk, wq_sb[:, cc, jg * 128:(jg + 1) * 128],
                                         strip[:, cc, :], start=(cc == 0), stop=(cc == 3))
                    nc.scalar.copy(qkvT[jg][:, nb * 512:(nb + 1) * 512], qk)

            # ---- LayerNorm stats over C (partition direction, via ones-matmuls) ----
            st_psum = psE.tile([128, M], F32, tag="p2", bufs=2)
            sq_psum = psE.tile([128, M], F32, tag="p2", bufs=2)
            y2 = [None] * 4
            for cc in range(4):
                y2t = early.tile([128, M], F32R, tag="ysq", bufs=1)
                nc.vector.tensor_mul(y2t, y[cc], y[cc])
                y2[cc] = y2t
            for mh in range(2):
                for cc in range(4):
                    nc.tensor.matmul(st_psum[0:1, mh * 512:(mh + 1) * 512], ones_col_r,
                                     y[cc][:, mh * 512:(mh + 1) * 512],
                                     start=(cc == 0), stop=(cc == 3))
                    nc.tensor.matmul(sq_psum[0:1, mh * 512:(mh + 1) * 512], ones_col_r,
                                     y2[cc][:, mh * 512:(mh + 1) * 512],
                                     start=(cc == 0), stop=(cc == 3))
            mu_sb = early.tile([1, M], F32R, tag="mu")
            nc.vector.tensor_scalar_mul(mu_sb, st_psum[0:1, :], 1.0 / C)
            ms_sb = early.tile([1, M], F32, tag="ms")
            nc.vector.tensor_scalar_mul(ms_sb, sq_psum[0:1, :], 1.0 / C)
            tmp_row = early.tile([1, M], F32, tag="tmp_row")
            nc.vector.tensor_mul(tmp_row, mu_sb, mu_sb)          # mu^2
            nc.vector.tensor_sub(ms_sb, ms_sb, tmp_row)          # var (in place)
            nc.scalar.activation(tmp_row, ms_sb, AF.Sqrt, bias=eps_sb[0:1, :], scale=1.0)
            rstd = early.tile([1, M], F32R, tag="rstd")
            nc.vector.reciprocal(rstd, tmp_row)

            # broadcast mu, rstd across partitions via K=1 matmuls
            bcp = psE.tile([128, M], F32, tag="p2", bufs=2)
            for mh in range(2):
                nc.tensor.matmul(bcp[:, mh * 512:(mh + 1) * 512], ones_row_r,
                                 mu_sb[:, mh * 512:(mh + 1) * 512], start=True, stop=True)
            nc.vector.tensor_copy(mu_bb, bcp)
            bcp2 = psE.tile([128, M], F32, tag="p2", bufs=2)
            for mh in range(2):
                nc.tensor.matmul(bcp2[:, mh * 512:(mh + 1) * 512], ones_row_r,
                                 rstd[:, mh * 512:(mh + 1) * 512], start=True, stop=True)
            nc.scalar.copy(rstd_bb, bcp2)

            # normalize + gamma/beta, then linear to x2 (duplicated rows)
            x2p = psE.tile([128, M], F32, tag="p2", bufs=2)
            for cc in range(4):
                t1 = early.tile([128, M], F32, tag="t1", bufs=2)
                nc.vector.tensor_sub(t1, y[cc], mu_bb)
                nc.vector.scalar_tensor_tensor(t1, t1, gam_sb[:, cc:cc + 1], rstd_bb,
                                               op0=OP.mult, op1=OP.mult)
                nc.scalar.activation(y[cc], t1, AF.Identity,
                                     bias=bet_sb[:, cc:cc + 1], scale=1.0)
            for mh in range(2):
                for cc in range(4):
                    nc.tensor.matmul(x2p[:, mh * 512:(mh + 1) * 512], wl_sb[:, cc, :],
                                     y[cc][:, mh * 512:(mh + 1) * 512],
                                     start=(cc == 0), stop=(cc == 3))
            for i in range(2):
                nc.scalar.activation(x2m[i], x2p, AF.Identity, bias=blin_sb, scale=1.0)
            nc.vector.tensor_copy(x2m[0][64:128, :], zf[64:128, :])
            nc.vector.tensor_copy(x2m[1][0:64, :], zf[0:64, :])

        # ================= P4-P5: token-major k/v, scores + pooling =================
        with tc.tile_pool(name="late", bufs=1) as late:
            psM_ctx = tc.tile_pool(name="psM", bufs=1, space="PSUM")
            psM = psM_ctx.__enter__()

            # token-major k/v with ones column: [128, 32 chunks, 65]
            kc_use = [late.tile([128, M], F32R, tag=f"kcu{h}", name=f"kcu{h}") for h in range(HL)]
            sscale = [late.tile([128, 8], F32, tag=f"ss{h}", name=f"ss{h}") for h in range(HL)]
            vtok = [late.tile([128, 8 * 66 + 62], F32R, tag=f"vt{h}", name=f"vt{h}") for h in range(HL)]

            for h in range(HL):
                base = 64 * (h % 2)
                kT = qkvT[2 + h // 2]
                vT = qkvT[4 + h // 2]
                idn = ident[base:base + 64, base:base + 64]

                ktok = late.tile([128, 32 * 66 + 62], F32R, tag="ktok", bufs=1)
                vtok_full = late.tile([128, 32 * 66 + 62], F32R, tag="vtokf", bufs=1)
                for src, dst in ((kT, ktok), (vT, vtok_full)):
                    dst3 = dst[:, 0:32 * 66].rearrange("p (c w) -> p c w", w=66)
                    for g8 in range(4):
                        tp2 = psM.tile([128, 512], F32R, tag="sc", bufs=2)
                        for t in range(8):
                            ncnk = g8 * 8 + t
                            nc.tensor.transpose(
                                tp2[:, t * 64:(t + 1) * 64],
                                src[base:base + 64, ncnk * 128:(ncnk + 1) * 128], idn)
                        ov = dst3[:, g8 * 8:(g8 + 1) * 8, 0:64]
                        nc.vector.tensor_copy(ov, tp2.rearrange("p (t d) -> p t d", t=8))
                    nc.vector.tensor_copy(
                        dst3[:, :, 64:66],
                        ones64_f.rearrange("p (a o) -> p a o", o=2))

                pool_k = psM.tile([128, M], F32, tag="poolk")
                pool_v = psM.tile([128, M], F32, tag="poolv")
                x2u = x2m[h % 2]
                for ncnk in range(32):
                    for src_tok, pool_ps, featT in ((ktok, pool_k, kT), (vtok_full, pool_v, vT)):
                        sc = psM.tile([128, M], F32, tag="sc", bufs=2)
                        for mh in range(2):
                            nc.tensor.matmul(sc[:, mh * 512:(mh + 1) * 512],
                                             featT[:, ncnk * 128:(ncnk + 1) * 128],
                                             x2u[:, mh * 512:(mh + 1) * 512],
                                             start=True, stop=True)
                        exps = late.tile([128, M], F32R, tag="expS", bufs=2)
                        nc.scalar.activation(exps, sc, AF.Exp)
                        for mh in range(2):
                            nc.tensor.matmul(pool_ps[:, mh * 512:(mh + 1) * 512],
                                             src_tok[:, ncnk * 66: ncnk * 66 + 128],
                                             exps[:, mh * 512:(mh + 1) * 512],
                                             start=(ncnk == 0), stop=(ncnk == 31))

                # evict pools; k_c into parity-aligned, zero-masked kc_use tile
                kc_ev = late.tile([128, M], F32R, tag="kctmp", bufs=1)
                nc.vector.tensor_copy(kc_ev[0:66, :], pool_k[0:66, :])
                if h % 2 == 0:
                    nc.vector.tensor_copy(kc_use[h][0:64, :], kc_ev[0:64, :])
                    nc.vector.tensor_copy(kc_use[h][64:128, :], zf[64:128, :])
                else:
                    nc.sync.dma_start(kc_use[h][64:128, :], kc_ev[0:64, :])
                    nc.vector.tensor_copy(kc_use[h][0:64, :], zf[0:64, :])
                vc_ev = late.tile([128, M], F32R, tag="vcev", bufs=1)
                nc.vector.tensor_copy(vc_ev[0:66, :], pool_v[0:66, :])

                # denominators: transpose-pack Dk|Dv rows -> [128, 16], recip
                dkp = psM.tile([128, 32], F32R, tag="sc", bufs=2)
                for mc in range(8):
                    nc.tensor.transpose(dkp[:, mc * 2:(mc + 1) * 2],
                                        kc_ev[64:66, mc * 128:(mc + 1) * 128],
                                        ident[64:66, 64:66])
                    nc.tensor.transpose(dkp[:, 16 + mc * 2:18 + mc * 2],
                                        vc_ev[64:66, mc * 128:(mc + 1) * 128],
                                        ident[64:66, 64:66])
                dd = late.tile([128, 32], F32, tag="dd", bufs=2)
                nc.vector.tensor_copy(dd, dkp)
                rr = late.tile([128, 32], F32, tag="rr", bufs=2)
                nc.vector.reciprocal(rr, dd)
                nc.vector.tensor_scalar_mul(sscale[h], rr[:, 0:16:2], SCALE)

                # vtok: transpose v_c rows, scale cols 0-63 by 1/Dv, ones col 64
                vtp = psM.tile([128, 528], F32R, tag="sc", bufs=2)
                for mc in range(8):
                    nc.tensor.transpose(vtp[:, mc * 66:(mc + 1) * 66],
                                        vc_ev[0:66, mc * 128:(mc + 1) * 128],
                                        ident[0:66, 0:66])
                vt3 = vtok[h][:, 0:8 * 66].rearrange("p (c w) -> p c w", w=66)
                for mc in range(8):
                    nc.vector.tensor_scalar_mul(vt3[:, mc, 0:64],
                                                vtp[:, mc * 66:mc * 66 + 64],
                                                rr[:, 16 + mc * 2:17 + mc * 2])
                nc.vector.tensor_copy(
                    vt3[:, :, 64:66],
                    ones64_f[:, 0:16].rearrange("p (a o) -> p a o", o=2))

            # ================= P6: attention =================
            psM_ctx.__exit__(None, None, None)
            with tc.tile_pool(name="psL", bufs=1, space="PSUM") as psL:
                for h in range(HL):
                    qT = qkvT[h // 2]
                    for nq in range(4):
                        outp = psL.tile([128, 1024], F32, tag="out", bufs=2)
                        for mc in range(8):
                            lg = psL.tile([128, 1024], F32, tag="lg", bufs=2)
                            for n2 in range(2):
                                nc.tensor.matmul(
                                    lg[:, n2 * 512:(n2 + 1) * 512],
                                    kc_use[h][:, mc * 128:(mc + 1) * 128],
                                    qT[:, nq * 1024 + n2 * 512: nq * 1024 + (n2 + 1) * 512],
                                    start=True, stop=True)
                            expl = late.tile([128, 1024], F32R, tag="expL", bufs=3)
                            nc.scalar.activation(expl, lg, AF.Exp,
                                                 bias=0.0, scale=sscale[h][:, mc:mc + 1])
                            for n2 in range(2):
                                nc.tensor.matmul(outp[:, n2 * 512:(n2 + 1) * 512],
                                                 vtok[h][:, mc * 66: mc * 66 + 128],
                                                 expl[:, n2 * 512:(n2 + 1) * 512],
                                                 start=(mc == 0), stop=(mc == 7))
                        # normalize by DL (row 64)
                        rdl = late.tile([1, 1024], F32R, tag="rdl", bufs=2)
                        nc.vector.reciprocal(rdl, outp[64:65, :])
                        bc2 = late.tile([64, 1024], F32R, tag="bc2", bufs=1)
                        nc.gpsimd.partition_broadcast(bc2, rdl)
                        ost = late.tile([64, 1024], F32R, tag="ost", bufs=2)
                        nc.vector.tensor_mul(ost, outp[0:64, :], bc2)
                        nc.sync.dma_start(oT_d[:, h, nq * 1024:(nq + 1) * 1024], ost)

        # ================= P7: output projection =================
        with tc.tile_pool(name="projp", bufs=1) as projp, \
             tc.tile_pool(name="psP", bufs=1, space="PSUM") as psP:
            for nb in range(32):
                ld = projp.tile([128, 2, 128], F32R, tag="ld", bufs=12)
                for g in range(2):
                    src = bass.AP(tensor=oT_d.tensor,
                                  offset=(2 * g) * N + nb * 128,
                                  ap=[[N, 2], [HL * N, 64], [1, 128]])
                    nc.sync.dma_start(ld[:, g, :].rearrange("p (a n) -> p a n", a=1), src)
                pj = psP.tile([128, 512], F32, tag="pj", bufs=4)
                for g in range(2):
                    nc.tensor.matmul(pj, ld[:, g, :], wp_sb[:, g, :],
                                     start=(g == 0), stop=(g == 1))
                o_sb = projp.tile([128, 512], F32, tag="osb", bufs=6)
                if nb % 2 == 0:
                    nc.vector.tensor_copy(o_sb, pj)
                else:
                    nc.scalar.copy(o_sb, pj)
                nc.sync.dma_start(out_d[nb * 128:(nb + 1) * 128, :], o_sb)

    nc.compile()
    return nc


def _get_nc():
    if "nc" not in _CACHE:
        _CACHE["nc"] = _build_nc()
    return _CACHE["nc"]


def _make_in_maps(x, w_qkv, w_proj, w_sr_conv, b_sr_conv,
                  sr_norm_scale, sr_norm_bias, w_sr_linear, b_sr_linear):
    f32 = np.float32
    x = np.ascontiguousarray(np.asarray(x, f32))
    wq4 = np.asarray(w_qkv, f32).reshape(C, 3, H, HD)
    wp3 = np.asarray(w_proj, f32).reshape(H, HD, C)
    wconv = np.ascontiguousarray(np.asarray(w_sr_conv, f32).reshape(C, 4))
    bconv = np.asarray(b_sr_conv, f32).reshape(C, 1)
    gam = np.asarray(sr_norm_scale, f32).reshape(C, 1)
    bet = np.asarray(sr_norm_bias, f32).reshape(C, 1)
    wlin = np.asarray(w_sr_linear, f32)                    # [C, 64]
    wlin2 = np.ascontiguousarray(np.concatenate([wlin, wlin], axis=1))   # [C, 128]
    blin = np.asarray(b_sr_linear, f32).reshape(HD, 1)
    blin2 = np.ascontiguousarray(np.concatenate([blin, blin], axis=0))   # [128, 1]

    in_maps = []
    for core in range(8):
        b = core // 2
        g = core % 2
        hs = slice(g * HL, (g + 1) * HL)
        wq_slice = np.ascontiguousarray(wq4[:, :, hs, :].reshape(C, 3 * HL * HD))
        wp_slice = np.ascontiguousarray(wp3[hs].reshape(HL * HD, C))
        in_maps.append(dict(
            x=np.ascontiguousarray(x[b]),
            wqkv=wq_slice,
            wconv=wconv,
            bconv=bconv,
            gam=gam,
            bet=bet,
            wlin=wlin2,
            blin=blin2,
            wproj=wp_slice,
        ))
    return in_maps


def kernel(x, w_qkv, w_proj, b_proj, w_sr_conv, b_sr_conv,
           sr_norm_scale, sr_norm_bias, w_sr_linear, b_sr_linear,
           _trace=False, _trace_kwargs=None):
    nc = _get_nc()
    in_maps = _make_in_maps(x, w_qkv, w_proj, w_sr_conv, b_sr_conv,
                            sr_norm_scale, sr_norm_bias, w_sr_linear, b_sr_linear)
    res = bass_utils.run_bass_kernel_spmd(
        nc, in_maps, list(range(8)), trace=_trace, **(_trace_kwargs or {}))
    bp = np.asarray(b_proj, np.float32)
    out = np.empty((B, N, C), np.float32)
    for b in range(B):
        out[b] = res.results[2 * b]["out"] + res.results[2 * b + 1]["out"] + bp
    if _trace:
        return out, res
    return out
